# revision 35
# baseline (speedup 1.0000x reference)
"""Patch TileContext._drain_and_barrier: this container's walrus codegen
rejects >2 sem waits on one CTRL (Drain) instruction. Split the kernel-tail
drain's waits across separate nop instructions (1 wait each)."""
import concourse.tile as tile  # noqa
import concourse.mybir as mybir
from concourse.vector_clock import ScopedClock
from concourse._compat import not_none as nn


def _drain_and_barrier_split(self, tick_clock, wait_clock):
    nc = self.nc
    carrier = nc.sync.nop()
    wait_clock.add_sem_waits(carrier.ins, ScopedClock({None: tick_clock.global_clock}))
    si = carrier.ins.sync_info
    waits = list(si.on_wait) if si and si.on_wait else []
    if len(waits) > 1:
        si.on_wait.clear()
        si.on_wait.append(waits[0])
        for w in waits[1:]:
            n2 = nc.sync.nop()
            n2.ins.sync_info = mybir.SyncInfo(on_wait=[w], on_update=[])
    nc.sync.drain()

    nc.all_engine_barrier()
    assert self.sems is not None
    popped = nc._tile_sem_poison_stack.pop()
    assert popped is self._sem_poison
    nc.clear_and_free_semaphores(list(self.sems.allocated().values()))
    nc.all_engine_barrier()


tile.TileContext._drain_and_barrier = _drain_and_barrier_split


# ---- global wait-splitting pass ----
# This walrus build packs at most MAX_WAITS sem-waits per instruction
# (ISA EVENTS struct holds one; codegen can prepend a limited number of
# sync-wait commands). Move excess waits onto InstNoOp carriers.
MAX_WAITS = 2

def fix_waits(nc, max_waits=MAX_WAITS):
    import concourse.mybir as mybir
    dma2 = getattr(nc, "_fix_dma_waits2", False)
    n_fixed = 0
    for fn in nc.m.functions:
        for blk in fn.blocks:
            insts = blk.instructions
            out = []
            for inst in insts:
                if isinstance(max_waits, dict):
                    lim = max_waits.get(getattr(inst.engine, "value", str(inst.engine)),
                                        max_waits.get("default", 1))
                else:
                    lim = max_waits
                if dma2 and isinstance(inst, mybir.InstDMACopy):
                    lim = 2
                si = getattr(inst, "sync_info", None)
                if si is not None and si.on_wait and len(si.on_wait) > lim:
                    waits = list(si.on_wait)
                    si.on_wait.clear()
                    for w in waits[:-lim] if lim else waits:
                        n_fixed += 1
                        nop = mybir.InstNoOp(
                            name=f"{inst.name}.wsplit{n_fixed}",
                            sync_info=mybir.SyncInfo(on_wait=[w], on_update=[]),
                            bass_nofuse=True,
                            engine=inst.engine,
                        )
                        out.append(nop)
                    for w in waits[-lim:] if lim else []:
                        si.on_wait.append(w)
                elif si is not None and si.on_wait and len(si.on_wait) > 1 and getattr(inst, "opcode", None) is None:
                    pass
                out.append(inst)
            blk.instructions = out
    return n_fixed


# auto-apply fix_waits on serialization
import concourse.bass as _bass
_orig_to_json_bytes = _bass.Bass.to_json_bytes

def _to_json_bytes_fixed(self, *a, **kw):
    try:
        fix_waits(self, max_waits=getattr(self, "_fix_max_waits", 1))
    except Exception as e:
        import traceback; traceback.print_exc()
    return _orig_to_json_bytes(self, *a, **kw)

_bass.Bass.to_json_bytes = _to_json_bytes_fixed


"""NodeModel GNN kernel for Trainium2 (Bass/Tile), 8-core SPMD. v4.

Strategy (host-packed edge blocks, zero indirect DMA):
- Shard destination NODES across 8 cores by degree rank (snake deal) so all
  cores share one degree-sorted column schedule with minimal padding. No
  collectives needed.
- Columns = destination nodes grouped by degree desc, tiles of 512 columns,
  round r = r-th edge per column. The HOST gathers per-edge endpoint
  features into a packed HBM array XB in exact schedule order; the device
  does only direct DMA loads + dense math (no indirect DMA, no transposes).
- Rounds processed in PAIRS; the odd round is padded to the even round's
  width with duplicate edges (idempotent for max/min; the sum skips pads).
- Layer 1 = fp8(e4m3) DoubleRow matmul (XB packed [64,2,S]); layers 2/3
  fp16. h3 pairs stack on 128 psum partitions (even rows 0:64, odd 64:128
  via matmul tile_position).
- Segment SUM accumulates on the PE in a persistent psum bank (one extra W3
  matmul per round, pad columns excluded by width) and is DMA'd out fp32.
  MAX/MIN: one fp32->fp16 copy per pair, then running fp16 SBUF max/min
  spread across DVE/Pool. Host folds partition halves, divides by degree,
  adds b3, un-permutes, and assembles the final concat (x and u[batch]
  passthrough).
"""

import numpy as np

import concourse.bass as bass
import concourse.tile as tile

F32 = mybir.dt.float32
F16 = mybir.dt.float16
F8 = mybir.dt.float8e4
I32 = mybir.dt.int32
AF = mybir.ActivationFunctionType
ALU = mybir.AluOpType

P = 128
W = 512  # tile width (columns = destination nodes)


def build_schedule(col, n_nodes, n_cores):
    """Host-side index preprocessing. Returns shared schedule + per-core arrays.
    Nodes are dealt to cores by degree rank (snake order) so every core's
    degree histogram is within 1 of the shared max histogram."""
    deg_all = np.bincount(col, minlength=n_nodes)
    dmax = int(deg_all.max())

    rank = np.argsort(-deg_all, kind="stable")  # nodes by degree desc
    node_core = np.empty(n_nodes, np.int64)
    snake = np.arange(2 * n_cores)
    snake = np.minimum(snake, 2 * n_cores - 1 - snake)  # 0..7,7..0
    node_core[rank] = snake[np.arange(n_nodes) % (2 * n_cores)]
    core_nodes = [np.where(node_core == c)[0] for c in range(n_cores)]

    hist = np.zeros((n_cores, dmax + 1), np.int64)
    for c in range(n_cores):
        hist[c] = np.bincount(deg_all[core_nodes[c]], minlength=dmax + 1)
    H = hist.max(axis=0)  # shared histogram (per exact degree), index 0 unused

    # shared column degree sequence, descending
    col_degs = np.repeat(np.arange(dmax, 0, -1), H[dmax:0:-1])
    n_cols = len(col_degs)
    n_tiles = (n_cols + W - 1) // W

    # CSR of edges by destination (stable order)
    order = np.argsort(col, kind="stable")
    starts = np.zeros(n_nodes + 1, np.int64)
    np.cumsum(deg_all, out=starts[1:])

    # per-core: map shared columns -> node ids (real) or -1 (virtual)
    col_node = np.full((n_cores, n_cols), -1, np.int64)
    for c in range(n_cores):
        own = core_nodes[c]
        d_own = deg_all[own]
        nodes_by_deg = {}
        for i in np.argsort(-d_own, kind="stable"):
            if d_own[i] == 0:
                break
            nodes_by_deg.setdefault(int(d_own[i]), []).append(int(own[i]))
        used = {d: 0 for d in range(1, dmax + 1)}
        for j in range(n_cols):
            d = int(col_degs[j])
            lst = nodes_by_deg.get(d, [])
            k = used[d]
            if k < len(lst):
                col_node[c, j] = lst[k]
                used[d] = k + 1

    # schedule: per tile, list of round widths; global column -> padded pos
    tiles = []
    col_pos = np.zeros(n_cols, np.int64)
    cc = 0
    for t in range(n_tiles):
        j0, j1 = t * W, min((t + 1) * W, n_cols)
        degs = col_degs[j0:j1]
        d_t = int(degs[0])
        widths = [int(np.searchsorted(-degs, -(r + 1), side="right")) for r in range(d_t)]
        tiles.append((j0, j1, widths, cc))
        col_pos[j0:j1] = cc * P + np.arange(j1 - j0)
        cc += (j1 - j0 + P - 1) // P

    return dict(
        deg_all=deg_all, col_degs=col_degs,
        n_cols=n_cols, n_tiles=n_tiles, tiles=tiles, order=order, starts=starts,
        col_node=col_node, col_pos=col_pos, n_col_chunks=cc, dmax=dmax,
    )


def build_pair_plan(sched):
    """Round-pair plan. Per tile: list of (w, w_n, w_nd, off, ho): w =
    even-round width, w_n = true odd-round width (0 if the odd round doesn't
    exist), w_nd = odd DEVICE width (w_n clamped up to >=8 with duplicate
    edges; host reads only the first w_n columns), off = global XB column
    offset of the even slab (odd slab at off+w, w_nd wide), ho = pair's h3
    output column offset."""
    plan = []
    off = 0
    ho = 0
    for (j0, j1, widths, cc0) in sched["tiles"]:
        d_t = len(widths)
        tp = []
        for r in range(0, d_t, 2):
            w = widths[r]
            w_n = widths[r + 1] if r + 1 < d_t else 0
            w_nd = 0 if w_n == 0 else min(w, max(w_n, 8))
            tp.append((w, w_n, w_nd, off, ho))
            off += w + w_nd
            ho += w
        plan.append(tp)
    return plan, off, ho  # totals: XB columns, h3 output columns


def make_in_maps(sched, plan, s_total, x, W1, W2, W3, b1, b2, n_cores, w1_mode="dr8"):
    """Per-core input dicts (shared program, per-core data)."""
    import ml_dtypes
    NP8 = ml_dtypes.float8_e4m3
    n_nodes = x.shape[0]
    tiles = sched["tiles"]
    col_node = sched["col_node"]
    col_degs = sched["col_degs"]
    order, starts = sched["order"], sched["starts"]
    row = sched["row"]

    xdt = NP8 if w1_mode == "dr8" else np.float16
    xz = np.zeros((n_nodes + 1, 64), xdt)
    xz[:n_nodes] = x.astype(xdt)

    in_maps = []
    for c in range(n_cores):
        nodes_all = col_node[c]
        # global index arrays into xz (n_nodes = zeros guard row)
        srcidx = np.full(s_total, n_nodes, np.int64)
        colidx = np.full(s_total, n_nodes, np.int64)
        for t, tp in enumerate(plan):
            j0, j1, widths, cc0 = tiles[t]
            nodes = nodes_all[j0:j1]
            degs = col_degs[j0:j1]
            for pi, (w, w_n, w_nd, off, ho) in enumerate(tp):
                r = 2 * pi
                narr = nodes[:w]
                real = narr >= 0
                nr = narr[real]
                # even slab: round r edge (always exists for real active cols)
                e = order[starts[nr] + r]
                srcidx[off:off + w][real] = row[e]
                colidx[off:off + w][real] = nr
                if w_nd:
                    # odd slab (device width): first w_n cols = true round r+1
                    # edges; the clamp pad [w_n:w_nd] duplicates round r
                    n2 = nodes[:w_nd]
                    real2 = n2 >= 0
                    nr2 = n2[real2]
                    rr = np.where(degs[:w_nd][real2] > r + 1, r + 1, r)
                    e2 = order[starts[nr2] + rr]
                    srcidx[off + w:off + w + w_nd][real2] = row[e2]
                    colidx[off + w:off + w + w_nd][real2] = nr2

        if w1_mode == "dr8":
            xb = np.empty((64, 2, s_total), NP8)
            xb[:, 0, :] = xz[srcidx].T
            xb[:, 1, :] = xz[colidx].T
            w1 = np.ascontiguousarray(
                np.stack([W1[:64], W1[64:]], axis=1).astype(NP8))  # [64,2,128]
        else:
            xb = np.empty((128, s_total), np.float16)
            xb[:64] = xz[srcidx].T
            xb[64:] = xz[colidx].T
            w1 = W1.astype(np.float16)

        in_maps.append({
            "XB": xb, "W1": w1,
            "W2": W2.astype(np.float16), "W3": W3.astype(np.float16),
            "b1": np.ascontiguousarray(b1[:, None].astype(np.float32)),
            "b2": np.ascontiguousarray(b2[:, None].astype(np.float32)),
            "tok": np.zeros((P, 1), np.float32),
        })
    return in_maps


def build_kernel(sched, plan, s_total, s_h, hid_ch=128, lat_ch=64, tune=None):
    """Emit the shared Bass program. tune["repeat"]>1 re-runs the whole tile
    loop (timing regression only; outputs simply overwritten)."""
    t = dict(w1_mode="dr8", relu1="act", relu2="dve", copy3="act,dve",
             max_waits=1, h1_bufs=3, h2_bufs=2, h3_bufs=1, h3t_bufs=2, xb_bufs=2, hsb_bufs=3,
             repeat=1, relu1_pair=0)
    t.update(tune or {})
    nc = bass.Bass()
    nc._fix_max_waits = t["max_waits"]
    tiles = sched["tiles"]
    ncc = sched["n_col_chunks"]
    s_max = max(tp[-1][3] + tp[-1][0] + tp[-1][2] - tp[0][3] for tp in plan)
    h_max = max(tp[-1][4] + tp[-1][0] - tp[0][4] for tp in plan)

    if t["w1_mode"] == "dr8":
        xb_t = nc.dram_tensor("XB", [64, 2, s_total], F8, kind="ExternalInput")
        w1_t = nc.dram_tensor("W1", [64, 2, hid_ch], F8, kind="ExternalInput")
    else:
        xb_t = nc.dram_tensor("XB", [128, s_total], F16, kind="ExternalInput")
        w1_t = nc.dram_tensor("W1", [128, hid_ch], F16, kind="ExternalInput")
    w2_t = nc.dram_tensor("W2", [hid_ch, hid_ch], F16, kind="ExternalInput")
    w3_t = nc.dram_tensor("W3", [hid_ch, lat_ch], F16, kind="ExternalInput")
    b1_t = nc.dram_tensor("b1", [hid_ch, 1], F32, kind="ExternalInput")
    b2_t = nc.dram_tensor("b2", [hid_ch, 1], F32, kind="ExternalInput")
    outH_t = nc.dram_tensor("outH", [128, s_h], F16, kind="ExternalOutput")
    tok_t = nc.dram_tensor("tok", [P, 1], F32, kind="ExternalInput")
    tokout_t = nc.dram_tensor("tok_out", [P, 1], F32, kind="ExternalOutput")

    def veng(name):
        return nc.vector if name == "dve" else nc.gpsimd

    def pick(spec, idx):
        """spec: 'eng' or 'e1,e2,...' rotated by idx."""
        parts = spec.split(",")
        return parts[idx % len(parts)]

    def relu_op(eng, out_ap, in_ap, bias):
        if eng == "act":
            nc.scalar.activation(out_ap, in_ap, AF.Relu, bias=bias)
        else:
            veng(eng).tensor_scalar(out=out_ap, in0=in_ap, scalar1=bias,
                                    scalar2=0.0, op0=ALU.add, op1=ALU.max)

    def copy_op(eng, out_ap, in_ap):
        if eng == "act":
            nc.scalar.activation(out_ap, in_ap, AF.Identity)
        else:
            veng(eng).tensor_copy(out_ap, in_ap)

    with tile.TileContext(nc) as tc:
        with (
            tc.tile_pool(name="const", bufs=1) as constp,
            tc.tile_pool(name="xb", bufs=t["xb_bufs"]) as xbp,
            tc.tile_pool(name="hsb", bufs=t["hsb_bufs"]) as hsbp,
            tc.tile_pool(name="h3sb", bufs=t["h3t_bufs"]) as h3sbp,
            tc.tile_pool(name="ps_h1", bufs=t["h1_bufs"], space="PSUM") as ps_h1,
            tc.tile_pool(name="ps_h2", bufs=t["h2_bufs"], space="PSUM") as ps_h2,
            tc.tile_pool(name="ps_h3", bufs=t["h3_bufs"], space="PSUM") as ps_h3,
        ):
            if t["w1_mode"] == "dr8":
                w1 = constp.tile([64, 2, hid_ch], F8)
            else:
                w1 = constp.tile([128, hid_ch], F16)
            nc.sync.dma_start(w1[:], w1_t[:])
            w2 = constp.tile([hid_ch, hid_ch], F16); nc.sync.dma_start(w2[:], w2_t[:])
            w3 = constp.tile([hid_ch, lat_ch], F16); nc.sync.dma_start(w3[:], w3_t[:])
            b1 = constp.tile([hid_ch, 1], F32); nc.sync.dma_start(b1[:], b1_t[:])
            b2 = constp.tile([hid_ch, 1], F32); nc.sync.dma_start(b2[:], b2_t[:])
            tok_sb = constp.tile([P, 1], F32)
            nc.sync.dma_start(tok_sb[:], tok_t[:])
            nc.sync.dma_start(tokout_t[:], tok_sb[:])

            n_t = len(plan)
            slabs = {}

            def load(ti):
                tp = plan[ti]
                off0 = tp[0][3]
                s_t = tp[-1][3] + tp[-1][0] + tp[-1][2] - off0
                cuts = [s_t]
                if ti == 0 and len(tp) > 2:
                    cuts = [tp[2][3] - off0, s_t]  # pairs 0-1 first, rest after
                if t["w1_mode"] == "dr8":
                    slab = xbp.tile([64, 2, s_max], F8, tag="slab")
                    a = 0
                    for c in cuts:
                        nc.sync.dma_start(slab[:, :, a:c], xb_t[:, :, off0 + a:off0 + c])
                        a = c
                else:
                    slab = xbp.tile([128, s_max], F16, tag="slab")
                    a = 0
                    for c in cuts:
                        nc.sync.dma_start(slab[:, a:c], xb_t[:, off0 + a:off0 + c])
                        a = c
                slabs[ti] = slab

            # flat round list: (ti, pair_idx, parity, width, w_nd, col_off)
            rounds = []
            for ti, tp in enumerate(plan):
                off0 = tp[0][3]
                for pi, (w, w_n, w_nd, offg, ho) in enumerate(tp):
                    o = offg - off0
                    rounds.append((ti, pi, 0, w, w_nd, o))
                    rounds.append((ti, pi, 1, w_nd, w_nd, o + w))

            rart = {}   # round idx -> h1p in flight
            hart = {}   # (ti, pi) -> h2p pair tile
            tctx = {}   # ti -> h3 tile buffer

            part1 = {}  # (ti, pi) -> h1p pair tile (relu1_pair mode)

            def stage1(ri):
                ti, pi, par, w, w_n, o = rounds[ri]
                if pi == 0 and par == 0 and ti + 1 < n_t:
                    load(ti + 1)
                if w == 0:
                    return
                slab = slabs[ti]
                if t["relu1_pair"]:
                    if par == 0:
                        h1p = ps_h1.tile([128, 2 * W], F32, tag="h1p")
                        part1[(ti, pi)] = h1p
                        dst = h1p[:, 0:w]
                    else:
                        dst = part1[(ti, pi)][:, W:W + w]
                else:
                    h1p = ps_h1.tile([128, W], F32, tag="h1p")
                    rart[ri] = h1p
                    dst = h1p[:, 0:w]
                if t["w1_mode"] == "dr8":
                    nc.tensor.matmul(out=dst, lhsT=w1[:],
                                     rhs=slab[:, :, o:o + w], start=True, stop=True,
                                     perf_mode=mybir.MatmulPerfMode.DoubleRow)
                else:
                    nc.tensor.matmul(out=dst, lhsT=w1[:],
                                     rhs=slab[:, o:o + w], start=True, stop=True)

            def stage2(ri):
                ti, pi, par, w, w_n, o = rounds[ri]
                if w == 0:
                    return
                if t["relu1_pair"]:
                    if par == 0:
                        return
                    h1p = part1.pop((ti, pi))
                    h1 = hsbp.tile([128, 2 * W], F16, tag="h1")
                    relu_op(pick(t["relu1"], pi), h1[:, 0:W + w], h1p[:, 0:W + w], b1[:])
                    h2p = ps_h2.tile([128, 2 * W], F32, tag="h2p")
                    hart[(ti, pi)] = h2p
                    nc.tensor.matmul(out=h2p[:, 0:w], lhsT=w2[:], rhs=h1[:, 0:w],
                                     start=True, stop=True)
                    nc.tensor.matmul(out=h2p[:, W:W + w], lhsT=w2[:], rhs=h1[:, W:W + w],
                                     start=True, stop=True)
                    return
                h1p = rart.pop(ri)
                h1 = hsbp.tile([128, W], F16, tag="h1")
                relu_op(pick(t["relu1"], ri), h1[:, 0:w], h1p[:, 0:w], b1[:])
                if par == 0:
                    h2p = ps_h2.tile([128, 2 * W], F32, tag="h2p")
                    hart[(ti, pi)] = h2p
                    nc.tensor.matmul(out=h2p[:, 0:w], lhsT=w2[:], rhs=h1[:, 0:w],
                                     start=True, stop=True)
                else:
                    h2p = hart[(ti, pi)]
                    nc.tensor.matmul(out=h2p[:, W:W + w], lhsT=w2[:], rhs=h1[:, 0:w],
                                     start=True, stop=True)

            def stage3(ri):
                ti, pi, par, w, w_n, o = rounds[ri]
                if par == 0:
                    return
                we = plan[ti][pi][0]
                h2p = hart.pop((ti, pi))
                h2 = hsbp.tile([128, 2 * W], F16, tag="h2")
                relu_op(pick(t["relu2"], pi), h2[:, 0:W + w_n], h2p[:, 0:W + w_n], b2[:])
                h3p = ps_h3.tile([128, W], F32, tag="h3p")
                nc.tensor.matmul(out=h3p[0:64, 0:we], lhsT=w3[:], rhs=h2[:, 0:we],
                                 start=True, stop=True)
                if w_n:
                    nc.tensor.matmul(out=h3p[64:128, 0:w_n], lhsT=w3[:],
                                     rhs=h2[:, W:W + w_n], start=True, stop=True)
                # evacuate psum as fp16 into the tile's SBUF h3 buffer; one
                # DMA per tile streams it to HBM; segment max/min/sum fold on
                # the host (odd half read at true odd width only)
                ho0 = plan[ti][0][4]
                if pi == 0:
                    h3t = h3sbp.tile([128, h_max], F16, tag="h3t")
                    tctx[ti] = h3t
                else:
                    h3t = tctx[ti]
                hoff = plan[ti][pi][4] - ho0
                copy_op(pick(t["copy3"], pi), h3t[:, hoff:hoff + we], h3p[:, :we])
                if pi == len(plan[ti]) - 1:
                    s_ht = hoff + we
                    nc.sync.dma_start(outH_t[:, ho0:ho0 + s_ht], h3t[:, :s_ht])
                    del tctx[ti]

            n_r = len(rounds)
            for _rep in range(t["repeat"]):
                load(0)
                for i in range(n_r + 2):
                    if i < n_r:
                        stage1(i)
                    if 0 <= i - 1 < n_r:
                        stage2(i - 1)
                    if 0 <= i - 2 < n_r:
                        stage3(i - 2)
    return nc


# ---------------- public entry point ----------------

N_NODES = 50000
N_EDGES = 800000
IN_CH = 64
HID_CH = 128
LAT_CH = 64
N_GRAPHS = 64
U_DIM = 32
N_CORES = 8


def assemble_output(sched, plan, res_list, x, u, batch, b3):
    """Host-side segment fold of streamed per-pair h3 blocks + concat."""
    n_nodes = x.shape[0]
    n_cols = sched["n_cols"]
    tiles = sched["tiles"]
    col_node = sched["col_node"]
    deg_all = sched["deg_all"]
    out = np.zeros((n_nodes, 288), np.float32)
    out[:, 0:64] = x
    out[:, 256:288] = u[batch]
    for c in range(N_CORES):
        outH = np.asarray(res_list[c]["outH"]).astype(np.float32)  # [128, S/2]
        vmax = np.full((64, n_cols), -np.inf, np.float32)
        vmin = np.full((64, n_cols), np.inf, np.float32)
        vsum = np.zeros((64, n_cols), np.float32)
        for ti, tp in enumerate(plan):
            j0 = tiles[ti][0]
            for pi, (w, w_n, w_nd, offg, ho) in enumerate(tp):
                h = outH[:, ho: ho + w]
                np.maximum(vmax[:, j0:j0 + w], h[0:64], out=vmax[:, j0:j0 + w])
                np.minimum(vmin[:, j0:j0 + w], h[0:64], out=vmin[:, j0:j0 + w])
                vsum[:, j0:j0 + w] += h[0:64]
                if w_n:
                    np.maximum(vmax[:, j0:j0 + w_n], h[64:128, :w_n],
                               out=vmax[:, j0:j0 + w_n])
                    np.minimum(vmin[:, j0:j0 + w_n], h[64:128, :w_n],
                               out=vmin[:, j0:j0 + w_n])
                    vsum[:, j0:j0 + w_n] += h[64:128, :w_n]
        nodes = col_node[c]
        real = nodes >= 0
        nds = nodes[real]
        d = deg_all[nds].astype(np.float32)
        out[nds, 64:128] = (vsum[:, real] / d).T
        out[nds, 128:192] = vmax[:, real].T
        out[nds, 192:256] = vmin[:, real].T
    nz = deg_all > 0
    out[nz, 64:256] += np.tile(b3, 3)[None, :]
    return out


def kernel(**inputs):
    """Full-input NodeModel forward. Returns [N_NODES, 288] float32."""
    from concourse.bass_utils import run_bass_kernel_spmd

    x = np.asarray(inputs["x"], np.float32)
    edge_index = np.asarray(inputs["edge_index"])
    u = np.asarray(inputs["u"], np.float32)
    batch = np.asarray(inputs["batch"])
    W1 = np.asarray(inputs["W1"], np.float32)
    b1 = np.asarray(inputs["b1"], np.float32)
    W2 = np.asarray(inputs["W2"], np.float32)
    b2 = np.asarray(inputs["b2"], np.float32)
    W3 = np.asarray(inputs["W3"], np.float32)
    b3 = np.asarray(inputs["b3"], np.float32)

    row = edge_index[0].astype(np.int32)
    col = edge_index[1].astype(np.int32)

    sched = build_schedule(col, x.shape[0], N_CORES)
    sched["row"] = row
    plan, s_total, s_h = build_pair_plan(sched)

    nc = build_kernel(sched, plan, s_total, s_h, W2.shape[0], W3.shape[1])
    in_maps = make_in_maps(sched, plan, s_total, x, W1, W2, W3, b1, b2, N_CORES)

    res = run_bass_kernel_spmd(nc, in_maps, core_ids=list(range(N_CORES)))
    return assemble_output(sched, plan, res.results, x, u, batch, b3).astype(np.float32)


# revision 38
# speedup vs baseline: 1.0382x; 1.0382x over previous
"""Patch TileContext._drain_and_barrier: this container's walrus codegen
rejects >2 sem waits on one CTRL (Drain) instruction. Split the kernel-tail
drain's waits across separate nop instructions (1 wait each)."""
import concourse.tile as tile  # noqa
import concourse.mybir as mybir
from concourse.vector_clock import ScopedClock
from concourse._compat import not_none as nn


def _drain_and_barrier_split(self, tick_clock, wait_clock):
    nc = self.nc
    carrier = nc.sync.nop()
    wait_clock.add_sem_waits(carrier.ins, ScopedClock({None: tick_clock.global_clock}))
    si = carrier.ins.sync_info
    waits = list(si.on_wait) if si and si.on_wait else []
    if len(waits) > 1:
        si.on_wait.clear()
        si.on_wait.append(waits[0])
        for w in waits[1:]:
            n2 = nc.sync.nop()
            n2.ins.sync_info = mybir.SyncInfo(on_wait=[w], on_update=[])
    nc.sync.drain()

    nc.all_engine_barrier()
    assert self.sems is not None
    popped = nc._tile_sem_poison_stack.pop()
    assert popped is self._sem_poison
    nc.clear_and_free_semaphores(list(self.sems.allocated().values()))
    nc.all_engine_barrier()


tile.TileContext._drain_and_barrier = _drain_and_barrier_split


# ---- global wait-splitting pass ----
# This walrus build packs at most MAX_WAITS sem-waits per instruction
# (ISA EVENTS struct holds one; codegen can prepend a limited number of
# sync-wait commands). Move excess waits onto InstNoOp carriers.
MAX_WAITS = 2

def fix_waits(nc, max_waits=MAX_WAITS):
    import concourse.mybir as mybir
    dma2 = getattr(nc, "_fix_dma_waits2", False)
    n_fixed = 0
    for fn in nc.m.functions:
        for blk in fn.blocks:
            insts = blk.instructions
            out = []
            for inst in insts:
                if isinstance(max_waits, dict):
                    lim = max_waits.get(getattr(inst.engine, "value", str(inst.engine)),
                                        max_waits.get("default", 1))
                else:
                    lim = max_waits
                if dma2 and isinstance(inst, mybir.InstDMACopy):
                    lim = 2
                si = getattr(inst, "sync_info", None)
                if si is not None and si.on_wait and len(si.on_wait) > lim:
                    waits = list(si.on_wait)
                    si.on_wait.clear()
                    for w in waits[:-lim] if lim else waits:
                        n_fixed += 1
                        nop = mybir.InstNoOp(
                            name=f"{inst.name}.wsplit{n_fixed}",
                            sync_info=mybir.SyncInfo(on_wait=[w], on_update=[]),
                            bass_nofuse=True,
                            engine=inst.engine,
                        )
                        out.append(nop)
                    for w in waits[-lim:] if lim else []:
                        si.on_wait.append(w)
                elif si is not None and si.on_wait and len(si.on_wait) > 1 and getattr(inst, "opcode", None) is None:
                    pass
                out.append(inst)
            blk.instructions = out
    return n_fixed


# auto-apply fix_waits on serialization
import concourse.bass as _bass
_orig_to_json_bytes = _bass.Bass.to_json_bytes

def _to_json_bytes_fixed(self, *a, **kw):
    try:
        fix_waits(self, max_waits=getattr(self, "_fix_max_waits", 1))
    except Exception as e:
        import traceback; traceback.print_exc()
    return _orig_to_json_bytes(self, *a, **kw)

_bass.Bass.to_json_bytes = _to_json_bytes_fixed


"""NodeModel GNN kernel for Trainium2 (Bass/Tile), 8-core SPMD. v7.

Strategy (host-packed edge blocks, zero indirect DMA):
- Shard destination NODES across 8 cores by degree rank (snake deal) so all
  cores share one degree-sorted column schedule with minimal padding. No
  collectives needed.
- Columns = destination nodes grouped by degree desc, tiles of 512 columns,
  round r = r-th edge per column. The HOST gathers per-edge endpoint
  features into a packed HBM array XB in exact schedule order; the device
  does only direct DMA loads + dense math (no indirect DMA, no transposes,
  no pad edges: odd-round slabs are true-width, clamped to >=8 columns
  because single-digit-width DoubleRow matmuls crash the exec unit).
- Layer 1 = fp8(e4m3) DoubleRow matmul (XB packed [64,2,S]); layers 2/3
  fp16. Pipeline per round pair: W1-DR -> relu1 (Act) -> W2 -> pair-packed
  relu2 (DVE) -> W3 stacked on 128 psum partitions (even rows 0:64, odd
  64:128 via matmul tile_position) -> one fp32->fp16 psum evacuation
  (Act/DVE alternating) into a per-tile SBUF buffer -> ONE DMA per tile to
  HBM (13 output DMAs total; DMA-instruction count dominates real-HW
  overhead far beyond what the cost model predicts).
- ALL segment reductions (max/min/sum over each node's edges) happen on the
  HOST from the streamed h3 blocks: fold partition halves, read the odd
  half at true width, divide by degree, add b3, un-permute, and assemble
  the final concat (x and u[batch] passthrough). Only Act/DVE may touch
  PSUM on TRN2 (GPSIMD cannot, and cannot run TensorTensor at all), so
  keeping reductions off-device leaves the two PSUM-capable engines for
  the irreducible relu/evacuation stream (~87% busy).
"""

import numpy as np

import concourse.bass as bass
import concourse.tile as tile

F32 = mybir.dt.float32
F16 = mybir.dt.float16
F8 = mybir.dt.float8e4
I32 = mybir.dt.int32
AF = mybir.ActivationFunctionType
ALU = mybir.AluOpType

P = 128
W = 512  # tile width (columns = destination nodes)


def build_schedule(col, n_nodes, n_cores):
    """Host-side index preprocessing. Returns shared schedule + per-core arrays.
    Nodes are dealt to cores by degree rank (snake order) so every core's
    degree histogram is within 1 of the shared max histogram."""
    deg_all = np.bincount(col, minlength=n_nodes)
    dmax = int(deg_all.max())

    rank = np.argsort(-deg_all, kind="stable")  # nodes by degree desc
    node_core = np.empty(n_nodes, np.int64)
    snake = np.arange(2 * n_cores)
    snake = np.minimum(snake, 2 * n_cores - 1 - snake)  # 0..7,7..0
    node_core[rank] = snake[np.arange(n_nodes) % (2 * n_cores)]
    core_nodes = [np.where(node_core == c)[0] for c in range(n_cores)]

    hist = np.zeros((n_cores, dmax + 1), np.int64)
    for c in range(n_cores):
        hist[c] = np.bincount(deg_all[core_nodes[c]], minlength=dmax + 1)
    H = hist.max(axis=0)  # shared histogram (per exact degree), index 0 unused

    # shared column degree sequence, descending
    col_degs = np.repeat(np.arange(dmax, 0, -1), H[dmax:0:-1])
    n_cols = len(col_degs)
    n_tiles = (n_cols + W - 1) // W

    # CSR of edges by destination (stable order)
    order = np.argsort(col, kind="stable")
    starts = np.zeros(n_nodes + 1, np.int64)
    np.cumsum(deg_all, out=starts[1:])

    # per-core: map shared columns -> node ids (real) or -1 (virtual)
    col_node = np.full((n_cores, n_cols), -1, np.int64)
    for c in range(n_cores):
        own = core_nodes[c]
        d_own = deg_all[own]
        nodes_by_deg = {}
        for i in np.argsort(-d_own, kind="stable"):
            if d_own[i] == 0:
                break
            nodes_by_deg.setdefault(int(d_own[i]), []).append(int(own[i]))
        used = {d: 0 for d in range(1, dmax + 1)}
        for j in range(n_cols):
            d = int(col_degs[j])
            lst = nodes_by_deg.get(d, [])
            k = used[d]
            if k < len(lst):
                col_node[c, j] = lst[k]
                used[d] = k + 1

    # schedule: per tile, list of round widths; global column -> padded pos
    tiles = []
    col_pos = np.zeros(n_cols, np.int64)
    cc = 0
    for t in range(n_tiles):
        j0, j1 = t * W, min((t + 1) * W, n_cols)
        degs = col_degs[j0:j1]
        d_t = int(degs[0])
        widths = [int(np.searchsorted(-degs, -(r + 1), side="right")) for r in range(d_t)]
        tiles.append((j0, j1, widths, cc))
        col_pos[j0:j1] = cc * P + np.arange(j1 - j0)
        cc += (j1 - j0 + P - 1) // P

    return dict(
        deg_all=deg_all, col_degs=col_degs,
        n_cols=n_cols, n_tiles=n_tiles, tiles=tiles, order=order, starts=starts,
        col_node=col_node, col_pos=col_pos, n_col_chunks=cc, dmax=dmax,
    )


def build_pair_plan(sched):
    """Round-pair plan. Per tile: list of (w, w_n, w_nd, off, ho): w =
    even-round width, w_n = true odd-round width (0 if the odd round doesn't
    exist), w_nd = odd DEVICE width (w_n clamped up to >=8 with duplicate
    edges; host reads only the first w_n columns), off = global XB column
    offset of the even slab (odd slab at off+w, w_nd wide), ho = pair's h3
    output column offset."""
    plan = []
    off = 0
    ho = 0
    for (j0, j1, widths, cc0) in sched["tiles"]:
        d_t = len(widths)
        tp = []
        for r in range(0, d_t, 2):
            w = widths[r]
            w_n = widths[r + 1] if r + 1 < d_t else 0
            w_nd = 0 if w_n == 0 else min(w, max(w_n, 8))
            tp.append((w, w_n, w_nd, off, ho))
            off += w + w_nd
            ho += w
        plan.append(tp)
    return plan, off, ho  # totals: XB columns, h3 output columns


def make_in_maps(sched, plan, s_total, x, W1, W2, W3, b1, b2, n_cores, w1_mode="dr8"):
    """Per-core input dicts (shared program, per-core data)."""
    import ml_dtypes
    NP8 = ml_dtypes.float8_e4m3
    n_nodes = x.shape[0]
    tiles = sched["tiles"]
    col_node = sched["col_node"]
    col_degs = sched["col_degs"]
    order, starts = sched["order"], sched["starts"]
    row = sched["row"]

    xdt = NP8 if w1_mode == "dr8" else np.float16
    xz = np.zeros((n_nodes + 1, 64), xdt)
    xz[:n_nodes] = x.astype(xdt)

    in_maps = []
    for c in range(n_cores):
        nodes_all = col_node[c]
        # global index arrays into xz (n_nodes = zeros guard row)
        srcidx = np.full(s_total, n_nodes, np.int64)
        colidx = np.full(s_total, n_nodes, np.int64)
        for t, tp in enumerate(plan):
            j0, j1, widths, cc0 = tiles[t]
            nodes = nodes_all[j0:j1]
            degs = col_degs[j0:j1]
            for pi, (w, w_n, w_nd, off, ho) in enumerate(tp):
                r = 2 * pi
                narr = nodes[:w]
                real = narr >= 0
                nr = narr[real]
                # even slab: round r edge (always exists for real active cols)
                e = order[starts[nr] + r]
                srcidx[off:off + w][real] = row[e]
                colidx[off:off + w][real] = nr
                if w_nd:
                    # odd slab (device width): first w_n cols = true round r+1
                    # edges; the clamp pad [w_n:w_nd] duplicates round r
                    n2 = nodes[:w_nd]
                    real2 = n2 >= 0
                    nr2 = n2[real2]
                    rr = np.where(degs[:w_nd][real2] > r + 1, r + 1, r)
                    e2 = order[starts[nr2] + rr]
                    srcidx[off + w:off + w + w_nd][real2] = row[e2]
                    colidx[off + w:off + w + w_nd][real2] = nr2

        if w1_mode == "dr8":
            xb = np.empty((64, 2, s_total), NP8)
            xb[:, 0, :] = xz[srcidx].T
            xb[:, 1, :] = xz[colidx].T
            w1 = np.ascontiguousarray(
                np.stack([W1[:64], W1[64:]], axis=1).astype(NP8))  # [64,2,128]
        else:
            xb = np.empty((128, s_total), np.float16)
            xb[:64] = xz[srcidx].T
            xb[64:] = xz[colidx].T
            w1 = W1.astype(np.float16)

        in_maps.append({
            "XB": xb, "W1": w1,
            "W2": W2.astype(np.float16), "W3": W3.astype(np.float16),
            "b1": np.ascontiguousarray(b1[:, None].astype(np.float32)),
            "b2": np.ascontiguousarray(b2[:, None].astype(np.float32)),
            "tok": np.zeros((P, 1), np.float32),
        })
    return in_maps


def build_kernel(sched, plan, s_total, s_h, hid_ch=128, lat_ch=64, tune=None):
    """Emit the shared Bass program. tune["repeat"]>1 re-runs the whole tile
    loop (timing regression only; outputs simply overwritten)."""
    t = dict(w1_mode="dr8", relu1="act", relu2="dve", copy3="act,dve",
             max_waits=1, h1_bufs=3, h2_bufs=2, h3_bufs=1, h3t_bufs=2, xb_bufs=2, hsb_bufs=3,
             repeat=1, relu1_pair=0)
    t.update(tune or {})
    nc = bass.Bass()
    nc._fix_max_waits = t["max_waits"]
    tiles = sched["tiles"]
    ncc = sched["n_col_chunks"]
    s_max = max(tp[-1][3] + tp[-1][0] + tp[-1][2] - tp[0][3] for tp in plan)
    h_max = max(tp[-1][4] + tp[-1][0] - tp[0][4] for tp in plan)

    if t["w1_mode"] == "dr8":
        xb_t = nc.dram_tensor("XB", [64, 2, s_total], F8, kind="ExternalInput")
        w1_t = nc.dram_tensor("W1", [64, 2, hid_ch], F8, kind="ExternalInput")
    else:
        xb_t = nc.dram_tensor("XB", [128, s_total], F16, kind="ExternalInput")
        w1_t = nc.dram_tensor("W1", [128, hid_ch], F16, kind="ExternalInput")
    w2_t = nc.dram_tensor("W2", [hid_ch, hid_ch], F16, kind="ExternalInput")
    w3_t = nc.dram_tensor("W3", [hid_ch, lat_ch], F16, kind="ExternalInput")
    b1_t = nc.dram_tensor("b1", [hid_ch, 1], F32, kind="ExternalInput")
    b2_t = nc.dram_tensor("b2", [hid_ch, 1], F32, kind="ExternalInput")
    outH_t = nc.dram_tensor("outH", [128, s_h], F16, kind="ExternalOutput")
    tok_t = nc.dram_tensor("tok", [P, 1], F32, kind="ExternalInput")
    tokout_t = nc.dram_tensor("tok_out", [P, 1], F32, kind="ExternalOutput")

    def veng(name):
        return nc.vector if name == "dve" else nc.gpsimd

    def pick(spec, idx):
        """spec: 'eng' or 'e1,e2,...' rotated by idx."""
        parts = spec.split(",")
        return parts[idx % len(parts)]

    def relu_op(eng, out_ap, in_ap, bias):
        if eng == "act":
            nc.scalar.activation(out_ap, in_ap, AF.Relu, bias=bias)
        else:
            veng(eng).tensor_scalar(out=out_ap, in0=in_ap, scalar1=bias,
                                    scalar2=0.0, op0=ALU.add, op1=ALU.max)

    def copy_op(eng, out_ap, in_ap):
        if eng == "act":
            nc.scalar.activation(out_ap, in_ap, AF.Identity)
        else:
            veng(eng).tensor_copy(out_ap, in_ap)

    with tile.TileContext(nc) as tc:
        with (
            tc.tile_pool(name="const", bufs=1) as constp,
            tc.tile_pool(name="xb", bufs=t["xb_bufs"]) as xbp,
            tc.tile_pool(name="hsb", bufs=t["hsb_bufs"]) as hsbp,
            tc.tile_pool(name="h3sb", bufs=t["h3t_bufs"]) as h3sbp,
            tc.tile_pool(name="ps_h1", bufs=t["h1_bufs"], space="PSUM") as ps_h1,
            tc.tile_pool(name="ps_h2", bufs=t["h2_bufs"], space="PSUM") as ps_h2,
            tc.tile_pool(name="ps_h3", bufs=t["h3_bufs"], space="PSUM") as ps_h3,
        ):
            if t["w1_mode"] == "dr8":
                w1 = constp.tile([64, 2, hid_ch], F8)
            else:
                w1 = constp.tile([128, hid_ch], F16)
            nc.sync.dma_start(w1[:], w1_t[:])
            w2 = constp.tile([hid_ch, hid_ch], F16); nc.sync.dma_start(w2[:], w2_t[:])
            w3 = constp.tile([hid_ch, lat_ch], F16); nc.sync.dma_start(w3[:], w3_t[:])
            b1 = constp.tile([hid_ch, 1], F32); nc.sync.dma_start(b1[:], b1_t[:])
            b2 = constp.tile([hid_ch, 1], F32); nc.sync.dma_start(b2[:], b2_t[:])
            tok_sb = constp.tile([P, 1], F32)
            nc.sync.dma_start(tok_sb[:], tok_t[:])
            nc.sync.dma_start(tokout_t[:], tok_sb[:])

            n_t = len(plan)
            slabs = {}

            def load(ti):
                tp = plan[ti]
                off0 = tp[0][3]
                s_t = tp[-1][3] + tp[-1][0] + tp[-1][2] - off0
                cuts = [s_t]
                if ti == 0 and len(tp) > 2:
                    cuts = [tp[2][3] - off0, s_t]  # pairs 0-1 first, rest after
                if t["w1_mode"] == "dr8":
                    slab = xbp.tile([64, 2, s_max], F8, tag="slab")
                    a = 0
                    for c in cuts:
                        nc.sync.dma_start(slab[:, :, a:c], xb_t[:, :, off0 + a:off0 + c])
                        a = c
                else:
                    slab = xbp.tile([128, s_max], F16, tag="slab")
                    a = 0
                    for c in cuts:
                        nc.sync.dma_start(slab[:, a:c], xb_t[:, off0 + a:off0 + c])
                        a = c
                slabs[ti] = slab

            # flat round list: (ti, pair_idx, parity, width, w_nd, col_off)
            rounds = []
            for ti, tp in enumerate(plan):
                off0 = tp[0][3]
                for pi, (w, w_n, w_nd, offg, ho) in enumerate(tp):
                    o = offg - off0
                    rounds.append((ti, pi, 0, w, w_nd, o))
                    rounds.append((ti, pi, 1, w_nd, w_nd, o + w))

            rart = {}   # round idx -> h1p in flight
            hart = {}   # (ti, pi) -> h2p pair tile
            tctx = {}   # ti -> h3 tile buffer

            part1 = {}  # (ti, pi) -> h1p pair tile (relu1_pair mode)

            def stage1(ri):
                ti, pi, par, w, w_n, o = rounds[ri]
                if pi == 0 and par == 0 and ti + 1 < n_t:
                    load(ti + 1)
                if w == 0:
                    return
                slab = slabs[ti]
                if t["relu1_pair"]:
                    if par == 0:
                        h1p = ps_h1.tile([128, 2 * W], F32, tag="h1p")
                        part1[(ti, pi)] = h1p
                        dst = h1p[:, 0:w]
                    else:
                        dst = part1[(ti, pi)][:, W:W + w]
                else:
                    h1p = ps_h1.tile([128, W], F32, tag="h1p")
                    rart[ri] = h1p
                    dst = h1p[:, 0:w]
                if t["w1_mode"] == "dr8":
                    nc.tensor.matmul(out=dst, lhsT=w1[:],
                                     rhs=slab[:, :, o:o + w], start=True, stop=True,
                                     perf_mode=mybir.MatmulPerfMode.DoubleRow)
                else:
                    nc.tensor.matmul(out=dst, lhsT=w1[:],
                                     rhs=slab[:, o:o + w], start=True, stop=True)

            def stage2(ri):
                ti, pi, par, w, w_n, o = rounds[ri]
                if w == 0:
                    return
                if t["relu1_pair"]:
                    if par == 0:
                        return
                    h1p = part1.pop((ti, pi))
                    h1 = hsbp.tile([128, 2 * W], F16, tag="h1")
                    relu_op(pick(t["relu1"], pi), h1[:, 0:W + w], h1p[:, 0:W + w], b1[:])
                    h2p = ps_h2.tile([128, 2 * W], F32, tag="h2p")
                    hart[(ti, pi)] = h2p
                    nc.tensor.matmul(out=h2p[:, 0:w], lhsT=w2[:], rhs=h1[:, 0:w],
                                     start=True, stop=True)
                    nc.tensor.matmul(out=h2p[:, W:W + w], lhsT=w2[:], rhs=h1[:, W:W + w],
                                     start=True, stop=True)
                    return
                h1p = rart.pop(ri)
                h1 = hsbp.tile([128, W], F16, tag="h1")
                relu_op(pick(t["relu1"], ri), h1[:, 0:w], h1p[:, 0:w], b1[:])
                if par == 0:
                    h2p = ps_h2.tile([128, 2 * W], F32, tag="h2p")
                    hart[(ti, pi)] = h2p
                    nc.tensor.matmul(out=h2p[:, 0:w], lhsT=w2[:], rhs=h1[:, 0:w],
                                     start=True, stop=True)
                else:
                    h2p = hart[(ti, pi)]
                    nc.tensor.matmul(out=h2p[:, W:W + w], lhsT=w2[:], rhs=h1[:, 0:w],
                                     start=True, stop=True)

            def stage3(ri):
                ti, pi, par, w, w_n, o = rounds[ri]
                if par == 0:
                    return
                we = plan[ti][pi][0]
                h2p = hart.pop((ti, pi))
                h2 = hsbp.tile([128, 2 * W], F16, tag="h2")
                relu_op(pick(t["relu2"], pi), h2[:, 0:W + w_n], h2p[:, 0:W + w_n], b2[:])
                h3p = ps_h3.tile([128, W], F32, tag="h3p")
                nc.tensor.matmul(out=h3p[0:64, 0:we], lhsT=w3[:], rhs=h2[:, 0:we],
                                 start=True, stop=True)
                if w_n:
                    nc.tensor.matmul(out=h3p[64:128, 0:w_n], lhsT=w3[:],
                                     rhs=h2[:, W:W + w_n], start=True, stop=True)
                # evacuate psum as fp16 into the tile's SBUF h3 buffer; one
                # DMA per tile streams it to HBM; segment max/min/sum fold on
                # the host (odd half read at true odd width only)
                ho0 = plan[ti][0][4]
                if pi == 0:
                    h3t = h3sbp.tile([128, h_max], F16, tag="h3t")
                    tctx[ti] = h3t
                else:
                    h3t = tctx[ti]
                hoff = plan[ti][pi][4] - ho0
                copy_op(pick(t["copy3"], pi), h3t[:, hoff:hoff + we], h3p[:, :we])
                if pi == len(plan[ti]) - 1:
                    s_ht = hoff + we
                    nc.sync.dma_start(outH_t[:, ho0:ho0 + s_ht], h3t[:, :s_ht])
                    del tctx[ti]

            n_r = len(rounds)
            for _rep in range(t["repeat"]):
                load(0)
                for i in range(n_r + 2):
                    if i < n_r:
                        stage1(i)
                    if 0 <= i - 1 < n_r:
                        stage2(i - 1)
                    if 0 <= i - 2 < n_r:
                        stage3(i - 2)
    return nc


# ---------------- public entry point ----------------

N_NODES = 50000
N_EDGES = 800000
IN_CH = 64
HID_CH = 128
LAT_CH = 64
N_GRAPHS = 64
U_DIM = 32
N_CORES = 8


def assemble_output(sched, plan, res_list, x, u, batch, b3):
    """Host-side segment fold of streamed per-pair h3 blocks + concat."""
    n_nodes = x.shape[0]
    n_cols = sched["n_cols"]
    tiles = sched["tiles"]
    col_node = sched["col_node"]
    deg_all = sched["deg_all"]
    out = np.zeros((n_nodes, 288), np.float32)
    out[:, 0:64] = x
    out[:, 256:288] = u[batch]
    for c in range(N_CORES):
        outH = np.asarray(res_list[c]["outH"]).astype(np.float32)  # [128, S/2]
        vmax = np.full((64, n_cols), -np.inf, np.float32)
        vmin = np.full((64, n_cols), np.inf, np.float32)
        vsum = np.zeros((64, n_cols), np.float32)
        for ti, tp in enumerate(plan):
            j0 = tiles[ti][0]
            for pi, (w, w_n, w_nd, offg, ho) in enumerate(tp):
                h = outH[:, ho: ho + w]
                np.maximum(vmax[:, j0:j0 + w], h[0:64], out=vmax[:, j0:j0 + w])
                np.minimum(vmin[:, j0:j0 + w], h[0:64], out=vmin[:, j0:j0 + w])
                vsum[:, j0:j0 + w] += h[0:64]
                if w_n:
                    np.maximum(vmax[:, j0:j0 + w_n], h[64:128, :w_n],
                               out=vmax[:, j0:j0 + w_n])
                    np.minimum(vmin[:, j0:j0 + w_n], h[64:128, :w_n],
                               out=vmin[:, j0:j0 + w_n])
                    vsum[:, j0:j0 + w_n] += h[64:128, :w_n]
        nodes = col_node[c]
        real = nodes >= 0
        nds = nodes[real]
        d = deg_all[nds].astype(np.float32)
        out[nds, 64:128] = (vsum[:, real] / d).T
        out[nds, 128:192] = vmax[:, real].T
        out[nds, 192:256] = vmin[:, real].T
    nz = deg_all > 0
    out[nz, 64:256] += np.tile(b3, 3)[None, :]
    return out


def kernel(**inputs):
    """Full-input NodeModel forward. Returns [N_NODES, 288] float32."""
    from concourse.bass_utils import run_bass_kernel_spmd

    x = np.asarray(inputs["x"], np.float32)
    edge_index = np.asarray(inputs["edge_index"])
    u = np.asarray(inputs["u"], np.float32)
    batch = np.asarray(inputs["batch"])
    W1 = np.asarray(inputs["W1"], np.float32)
    b1 = np.asarray(inputs["b1"], np.float32)
    W2 = np.asarray(inputs["W2"], np.float32)
    b2 = np.asarray(inputs["b2"], np.float32)
    W3 = np.asarray(inputs["W3"], np.float32)
    b3 = np.asarray(inputs["b3"], np.float32)

    row = edge_index[0].astype(np.int32)
    col = edge_index[1].astype(np.int32)

    sched = build_schedule(col, x.shape[0], N_CORES)
    sched["row"] = row
    plan, s_total, s_h = build_pair_plan(sched)

    nc = build_kernel(sched, plan, s_total, s_h, W2.shape[0], W3.shape[1])
    in_maps = make_in_maps(sched, plan, s_total, x, W1, W2, W3, b1, b2, N_CORES)

    res = run_bass_kernel_spmd(nc, in_maps, core_ids=list(range(N_CORES)))
    return assemble_output(sched, plan, res.results, x, u, batch, b3).astype(np.float32)
